# revision 8
# baseline (speedup 1.0000x reference)
import sys

sys.path.insert(0, "/opt/trn_rl_repo")

from contextlib import ExitStack

import numpy as np
import ml_dtypes

import concourse.bacc as bacc
import concourse.mybir as mybir
from concourse import tile
from concourse.bass_utils import run_bass_kernel_spmd

F32 = mybir.dt.float32
BF16 = mybir.dt.bfloat16
AL = mybir.AluOpType
AF = mybir.ActivationFunctionType
BF_NP = ml_dtypes.bfloat16

C = 256
H = W = 64
NC = 8  # cores / batch shards


# ---------------------------------------------------------------- host prep
def host_prep(inp):
    """Rearrange weights into [partition, free] layouts matching SBUF tiles."""
    d = {}
    f = np.float32

    # conditioning nets (dsc1, dsc2) — identical to baseline, f32
    for i, pre in ((0, "dsc1"), (1, "dsc2")):
        w1 = np.asarray(inp[f"{pre}_w1"], f)  # [64, 256]
        b1 = np.asarray(inp[f"{pre}_b1"], f)  # [64]
        w2 = np.asarray(inp[f"{pre}_w2"], f)  # [2304, 64]
        b2 = np.asarray(inp[f"{pre}_b2"], f)  # [2304]
        # lhsT for gm matmul: [k_local, chunk, m]; fold the 1/(H*W) mean here
        d[f"w1T{i}"] = np.ascontiguousarray(
            (w1.T / (H * W)).reshape(2, 128, 64).transpose(1, 0, 2)
        ).reshape(128, 128)
        d[f"b1_{i}"] = b1.reshape(64, 1).copy()
        GAM = 0.7978845608028654
        d[f"b1c{i}"] = (0.044715 * GAM * b1).reshape(64, 1).astype(f)
        d[f"b1g{i}"] = (GAM * b1).reshape(64, 1).astype(f)
        # lhsT for wts matmul: [j, chunk, k, c_local]; fold gelu's 0.5 here.
        # row 64 carries b2 so the bias rides the matmul (rhs row 64 == 1).
        w2ra = np.zeros((65, 2304), f)
        w2ra[:64] = np.ascontiguousarray(
            (0.5 * w2).reshape(2, 128, 9, 64).transpose(3, 0, 2, 1)
        ).reshape(64, 2304)
        w2ra[64] = np.ascontiguousarray(
            b2.reshape(2, 128, 9).transpose(0, 2, 1)).reshape(2304)
        d[f"w2r{i}"] = w2ra

    # identity for on-device diag(w) construction
    d["ident"] = np.eye(128, dtype=BF_NP)

    # channel_align 1x1: [k_local, kc, mc, m] bf16
    aw = np.asarray(inp["align_w"], f)[:, :, 0, 0]  # [256, 512]
    d["alignw"] = np.ascontiguousarray(
        aw.reshape(2, 128, 4, 128).transpose(3, 2, 0, 1)
    ).reshape(128, 1024).astype(BF_NP)
    d["alignb"] = np.ascontiguousarray(
        np.asarray(inp["align_b"], f).reshape(2, 128).T
    )  # [128, 2]

    # composed up conv: up2 o pixel_shuffle o up1 -> per-phase 3x3, 256->128
    # up1 channel index = 4c + p (p = 2r+s); cw_p = up2w @ W1[p::4]
    w1c = np.asarray(inp["up_w1"], f)          # [1024, 256, 3, 3]
    b1c = np.asarray(inp["up_b1"], f)          # [1024]
    u2 = np.asarray(inp["up_w2"], f)[:, :, 0, 0]  # [128, 256]
    b2c = np.asarray(inp["up_b2"], f)          # [128]
    cw = np.zeros((128, 4, 9, 2, 128), f)      # [k_local, p, tap, kc, m]
    cb = np.zeros((128, 4), f)
    for p in range(4):
        wp = np.tensordot(u2, w1c[p::4], axes=([1], [0]))  # [128m, 256k, 3, 3]
        t = wp.reshape(128, 2, 128, 3, 3).transpose(2, 3, 4, 1, 0)  # [kl,dy,dx,kc,m]
        cw[:, p] = t.reshape(128, 9, 2, 128)
        cb[:, p] = b2c + u2 @ b1c[p::4]
    d["cw"] = np.ascontiguousarray(cw).reshape(128, 4 * 9 * 2 * 128).astype(BF_NP)
    d["cb"] = cb

    # ---- polyphase re_enhance (same mapping as baseline, bf16 weights) ----
    def split(v):  # v = r + dy - 1
        rp = v % 2
        return rp, (v - rp) // 2

    r1w = np.asarray(inp["re_w1"], f)  # [32, 128, 3, 3]
    keymap = {}
    for p in range(4):
        r, s = p // 2, p % 2
        for dy in range(3):
            for dx in range(3):
                rp, qy = split(r + dy - 1)
                sp, qx = split(s + dx - 1)
                keymap.setdefault((2 * rp + sp, qy, qx), []).append((p, dy, dx))
    keys = sorted(keymap.keys(), key=lambda k: (k[1] != 0 or k[2] != 0, k))
    re1_keys = keys  # list of (p_in, qy, qx)
    re1w = np.zeros((128, 16, 128), f)
    for ki, key in enumerate(keys):
        for (p, dy, dx) in keymap[key]:
            re1w[:, ki, p * 32: (p + 1) * 32] = r1w[:, :, dy, dx].T
    d["re1w"] = re1w.reshape(128, 2048).astype(BF_NP)
    d["re1b"] = np.tile(np.asarray(inp["re_b1"], f), 4).reshape(128, 1)

    r2w = np.asarray(inp["re_w2"], f)  # [128, 32, 3, 3]
    re2_q = []  # per out-phase list of shifts, (0,0) first
    re2w = np.zeros((128, 4, 4, 128), f)
    for p in range(4):
        r, s = p // 2, p % 2
        qys = sorted({split(r + dy - 1)[1] for dy in range(3)}, key=lambda q: q != 0)
        qxs = sorted({split(s + dx - 1)[1] for dx in range(3)}, key=lambda q: q != 0)
        qs = [(qy, qx) for qy in qys for qx in qxs]
        qs.sort(key=lambda q: q != (0, 0))
        re2_q.append(qs)
        for qi, (qy, qx) in enumerate(qs):
            for pp in range(4):
                rp, sp = pp // 2, pp % 2
                dy = 2 * qy + rp - r + 1
                dx = 2 * qx + sp - s + 1
                if 0 <= dy < 3 and 0 <= dx < 3:
                    re2w[pp * 32: (pp + 1) * 32, p, qi, :] = r2w[:, :, dy, dx].T
    d["re2w"] = re2w.reshape(128, 2048).astype(BF_NP)
    d["re2b"] = np.asarray(inp["re_b2"], f).reshape(128, 1).copy()

    return d, re1_keys, re2_q


RE1_KEYS = None
RE2_Q = None


def _mapping():
    global RE1_KEYS, RE2_Q
    if RE1_KEYS is None:
        zeros = {k: np.zeros(v) for k, v in [
            ("dsc1_w1", (64, 256)), ("dsc1_b1", (64,)), ("dsc1_w2", (2304, 64)),
            ("dsc1_b2", (2304,)), ("dsc2_w1", (64, 256)), ("dsc2_b1", (64,)),
            ("dsc2_w2", (2304, 64)), ("dsc2_b2", (2304,)),
            ("align_w", (256, 512, 1, 1)), ("align_b", (256,)),
            ("up_w1", (1024, 256, 3, 3)), ("up_b1", (1024,)),
            ("up_w2", (128, 256, 1, 1)), ("up_b2", (128,)),
            ("re_w1", (32, 128, 3, 3)), ("re_b1", (32,)),
            ("re_w2", (128, 32, 3, 3)), ("re_b2", (128,)),
        ]}
        _, RE1_KEYS, RE2_Q = host_prep(zeros)
    return RE1_KEYS, RE2_Q


# ---------------------------------------------------------------- bass build
def pad2(ap):
    """View of a [128, 66*66] tile as [128, 66, 66]; image lives at [1:65,1:65]."""
    return ap.rearrange("p (y x) -> p y x", y=66)


def build():
    re1_keys, re2_q = _mapping()
    nc = bacc.Bacc(trn_type="TRN2", target_bir_lowering=False, debug=False)

    # x shipped host-padded to 66 cols (zeros at x=0,65): [256, 64*66]
    x_d = [nc.dram_tensor(n, [256, 4224], BF16, kind="ExternalInput")
           for n in ("x1", "x2")]
    wd = {}
    for name, shape, dt in [
        ("w1T0", [128, 128], F32), ("w1T1", [128, 128], F32),
        ("b1_0", [64, 1], F32), ("b1_1", [64, 1], F32),
        ("b1c0", [64, 1], F32), ("b1c1", [64, 1], F32),
        ("b1g0", [64, 1], F32), ("b1g1", [64, 1], F32),
        ("w2r0", [65, 2304], F32), ("w2r1", [65, 2304], F32),
        ("ident", [128, 128], BF16),
        ("alignw", [128, 1024], BF16), ("alignb", [128, 2], F32),
        ("cw", [128, 9216], BF16), ("cb", [128, 4], F32),
        ("re1w", [128, 2048], BF16), ("re1b", [128, 1], F32),
        ("re2w", [128, 2048], BF16), ("re2b", [128, 1], F32),
    ]:
        wd[name] = nc.dram_tensor(name, shape, dt, kind="ExternalInput")
    out_d = nc.dram_tensor("out", [128, 16384], F32, kind="ExternalOutput")

    with tile.TileContext(nc) as tc, ExitStack() as ctx:
        wpool = ctx.enter_context(tc.tile_pool(name="w", bufs=1))
        big = ctx.enter_context(tc.tile_pool(name="big", bufs=12))
        bnd = ctx.enter_context(tc.tile_pool(name="bnd", bufs=4))
        stg = ctx.enter_context(tc.tile_pool(name="stg", bufs=6))
        dpool = ctx.enter_context(tc.tile_pool(name="diag", bufs=6))
        tmp = ctx.enter_context(tc.tile_pool(name="tmp", bufs=2))
        tiny = ctx.enter_context(tc.tile_pool(name="tiny", bufs=6))
        ps = ctx.enter_context(tc.tile_pool(name="ps", bufs=7, space="PSUM"))
        psc = ctx.enter_context(tc.tile_pool(name="psc", bufs=1, space="PSUM"))

        wt = {}

        def load_w(names):
            for name in names:
                t = wpool.tile(list(wd[name].shape), wd[name].dtype, tag=name)
                nc.sync.dma_start(t[:], wd[name].ap())
                wt[name] = t

        def new_img(tag, borders=True):
            t = big.tile([128, 4356], BF16, tag="big")
            if borders:
                v = pad2(t[:])
                nc.gpsimd.memset(v[:, 0:66:65, :], 0.0)
                nc.gpsimd.memset(v[:, 1:65, 0:66:65], 0.0)
            return t

        # x in first (head latency), cond weights next, big weights later
        xin = [[], []]
        gms1 = [[None, None], [None, None]]
        scratch = big.tile([128, 4356], BF16, tag="big")

        def load_x(i, c, piece=None, npieces=2):
            if piece in (None, 0):
                t = big.tile([128, 4356], BF16, tag="big")
                v = pad2(t[:])
                nc.gpsimd.memset(v[:, 0:66:65, :], 0.0)  # rows 0,65
                xin[i].append(t)
            t = xin[i][c]
            if piece is None:
                nc.sync.dma_start(t[:, 66: 66 + 4224],
                                  x_d[i].ap()[c * 128: (c + 1) * 128, :])
            else:
                o0 = piece * 4224 // npieces
                o1 = (piece + 1) * 4224 // npieces
                nc.sync.dma_start(t[:, 66 + o0: 66 + o1],
                                  x_d[i].ap()[c * 128: (c + 1) * 128, o0: o1])

        # interleave tiny cond weights between x1 half-DMAs so nothing gates
        load_x(0, 0, 0)
        load_w(["w1T0", "b1_0"])
        load_x(0, 0, 1)
        load_x(0, 1, 0, npieces=3)
        load_x(0, 1, 1, npieces=3)
        load_x(0, 1, 2, npieces=3)
        load_w(["b1c0", "b1g0", "w2r0", "ident"])
        load_x(1, 0)
        load_x(1, 1)
        load_w(["w1T1", "b1_1", "b1c1", "b1g1", "w2r1",
                "alignw", "alignb", "re1b", "re2b", "cb"])

        def gm_of(i):
            for c in range(2):
                g = tiny.tile([128, 1], F32, tag="gms")
                t = xin[i][c]
                if (i, c) == (0, 0):
                    # halves on Act + DVE in parallel
                    gp = tiny.tile([128, 2], F32, tag="gmp")
                    nc.scalar.activation(scratch[:, :2178], t[:, :2178], AF.Copy,
                                         accum_out=gp[:, 0:1])
                    nc.vector.tensor_reduce(gp[:, 1:2], t[:, 2178:],
                                            axis=mybir.AxisListType.X, op=AL.add)
                    nc.vector.tensor_reduce(g[:], gp[:],
                                            axis=mybir.AxisListType.X, op=AL.add)
                elif (i, c) == (0, 1):
                    # critical chunk: thirds matching the DMA pieces (Act/DVE/Act)
                    b0, b1 = 66 + 1408, 66 + 2816
                    gp = tiny.tile([128, 3], F32, tag="gmp")
                    nc.scalar.activation(scratch[:, :b0], t[:, :b0], AF.Copy,
                                         accum_out=gp[:, 0:1])
                    nc.vector.tensor_reduce(gp[:, 1:2], t[:, b0:b1],
                                            axis=mybir.AxisListType.X, op=AL.add)
                    nc.scalar.activation(scratch[:, b1:], t[:, b1:], AF.Copy,
                                         accum_out=gp[:, 2:3])
                    nc.vector.tensor_reduce(g[:], gp[:],
                                            axis=mybir.AxisListType.X, op=AL.add)
                elif c == 0:
                    nc.vector.tensor_reduce(g[:], t[:], axis=mybir.AxisListType.X,
                                            op=AL.add)
                else:
                    nc.scalar.activation(scratch[:], t[:], AF.Copy, accum_out=g[:])
                gms1[i][c] = g

        # ---------------- conditioning (same math as baseline) ----------------
        def conditioning(d, gms):
            """gms: two [128,1] sum tiles -> per-chunk (wts [128,9], w4p [128,1])."""
            pgt = psc.tile([128, 9], F32, tag="psc")
            pg = pgt[:64, 0:1]
            for c in range(2):
                nc.tensor.matmul(pg, wt[f"w1T{d}"][:, c * 64: (c + 1) * 64],
                                 gms[c][:], start=(c == 0), stop=(c == 1))
            GAM = 0.7978845608028654
            u = tiny.tile([64, 1], F32, tag="u")
            nc.scalar.activation(u[:], pg, AF.Identity, bias=wt[f"b1_{d}"][:])
            sq = tiny.tile([64, 1], F32, tag="sq")
            nc.scalar.activation(sq[:], pg, AF.Square, bias=wt[f"b1_{d}"][:])
            uc = tiny.tile([64, 1], F32, tag="uc")
            nc.scalar.activation(uc[:], pg, AF.Identity, scale=0.044715 * GAM,
                                 bias=wt[f"b1c{d}"][:])
            ug = tiny.tile([64, 1], F32, tag="ug")
            nc.scalar.activation(ug[:], pg, AF.Identity, scale=GAM,
                                 bias=wt[f"b1g{d}"][:])
            v3 = tiny.tile([64, 1], F32, tag="v3")
            nc.scalar.activation(v3[:], sq[:], AF.Copy, scale=uc[:])
            th = tiny.tile([64, 1], F32, tag="th")
            nc.scalar.activation(th[:], v3[:], AF.Tanh, bias=ug[:])
            hv = tiny.tile([65, 1], F32, tag="hv")
            nc.vector.memset(hv[64:65, :], 1.0)
            nc.scalar.activation(hv[:64, :], th[:], AF.Identity, scale=u[:],
                                 bias=u[:])

            res = []
            for c in range(2):
                pw = psc.tile([128, 9], F32, tag="psc")
                for k in range(9):
                    nc.tensor.matmul(pw[:, k: k + 1],
                                     wt[f"w2r{d}"][:, (c * 9 + k) * 128: (c * 9 + k + 1) * 128],
                                     hv[:], start=True, stop=True)
                ex = tiny.tile([128, 9], F32, tag="ex")
                ssum = tiny.tile([128, 1], F32, tag="ssum")
                nc.scalar.activation(ex[:], pw[:], AF.Exp, accum_out=ssum[:])
                exc = tiny.tile([128, 1], F32, tag="exc")
                nc.vector.tensor_tensor(exc[:], ssum[:], ex[:, 4:5], AL.add)
                rec = tiny.tile([128, 1], F32, tag="rec")
                nc.vector.reciprocal(rec[:], ssum[:])
                wts = tiny.tile([128, 9], F32, tag="wts")
                nc.vector.tensor_scalar_mul(wts[:], ex[:], rec[:])
                w4p = tiny.tile([128, 1], F32, tag="w4p")
                nc.vector.tensor_scalar_add(w4p[:], wts[:, 4:5], 1.0)
                res.append((wts, w4p, ex, exc, rec))
            return res

        # ---------------- depthwise pieces ----------------
        def make_diag(cnd):
            # diag carries unnormalized softmax numerators (exp); the 1/sum
            # normalization rides the psum evacuation's scale instead.
            wts, w4p, ex, exc, rec = cnd
            diag = dpool.tile([128, 1152], BF16, tag="diag")
            for k in range(9):
                sc = exc if k == 4 else ex[:, k: k + 1]
                nc.scalar.activation(diag[:, k * 128: (k + 1) * 128], wt["ident"][:],
                                     AF.Copy, scale=sc)
            return diag

        def dw_rows(src, dst, cnd, diag, bands, relu=False, gacc=None, gi=0,
                    defer=None):
            """Emit depthwise for row bands: list of (lane, r0, r1), lane in
            {"pe","dve","pool"}; pe rows must be 8-aligned. relu: finalize with
            ReLU, accumulating partials into gacc columns starting at gi."""
            wts, w4p, ex, exc, rec = cnd
            vs, vd = pad2(src[:]), pad2(dst[:])
            vec_ranges = []
            for lane, r0, r1 in bands:
                if r1 <= r0:
                    continue
                if lane == "pe":
                    for a in range(r0, r1, 8):
                        p = ps.tile([128, 512], F32, tag="ps")
                        for k in range(9):
                            sy, sx = k // 3 - 1, k % 3 - 1
                            rhs = vs[:, 1 + a + sy: 9 + a + sy, 1 + sx: 65 + sx]
                            nc.tensor.matmul(p[:], diag[:, k * 128: (k + 1) * 128],
                                             rhs, start=(k == 0), stop=(k == 8))
                        kw = {}
                        if relu:
                            kw["accum_out"] = gacc[:, gi: gi + 1]
                            gi += 1
                        nc.scalar.activation(vd[:, 1 + a: 9 + a, 1: 65],
                                             p[:].rearrange("p (y x) -> p y x", y=8),
                                             AF.Relu if relu else AF.Identity,
                                             scale=rec[:], **kw)
                elif lane == "dve":
                    # center via tensor_scalar (4x), taps via ts(4x)+tt(2x)
                    nr = r1 - r0
                    dv = vd[:, 1 + r0: 1 + r1, 1: 65]
                    nc.vector.tensor_scalar(dv, vs[:, 1 + r0: 1 + r1, 1: 65],
                                            w4p[:], None, AL.mult)
                    for k in range(9):
                        if k == 4:
                            continue
                        sy, sx = k // 3 - 1, k % 3 - 1
                        t = tmp.tile([128, 2048], BF16, tag="tmp")
                        tv = t[:, : nr * 64].rearrange("p (y x) -> p y x", x=64)
                        nc.vector.tensor_scalar(
                            tv, vs[:, 1 + r0 + sy: 1 + r1 + sy, 1 + sx: 65 + sx],
                            wts[:, k: k + 1], None, AL.mult)
                        nc.vector.tensor_tensor(dv, dv, tv, AL.add)
                    vec_ranges.append((r0, r1))
                else:  # pair lane: DVE prescale (4x tensor_scalar) + Pool add
                    nr = r1 - r0
                    dv = vd[:, 1 + r0: 1 + r1, 1: 65]
                    nc.vector.tensor_scalar(dv, vs[:, 1 + r0: 1 + r1, 1: 65],
                                            w4p[:], None, AL.mult)
                    for k in range(9):
                        if k == 4:
                            continue
                        sy, sx = k // 3 - 1, k % 3 - 1
                        t = tmp.tile([128, 2048], BF16, tag="tmp2")
                        tv = t[:, : nr * 64].rearrange("p (y x) -> p y x", x=64)
                        nc.vector.tensor_scalar(
                            tv, vs[:, 1 + r0 + sy: 1 + r1 + sy, 1 + sx: 65 + sx],
                            wts[:, k: k + 1], None, AL.mult)
                        nc.gpsimd.tensor_tensor(dv, dv, tv, AL.add)
                    vec_ranges.append((r0, r1))
            if relu and vec_ranges:
                lo = min(r[0] for r in vec_ranges)
                hi = max(r[1] for r in vec_ranges)
                if defer is not None:
                    defer.append((dst, lo, hi, gacc, gi))
                else:
                    dv = vd[:, 1 + lo: 1 + hi, 1: 65]
                    nc.scalar.activation(dv, dv, AF.Relu,
                                         accum_out=gacc[:, gi: gi + 1])
                gi += 1
            return gi

        DW1_X1 = [("pe", 0, 40), ("dve", 40, 60), ("pool", 60, 64)]
        DW1_X2 = [("pe", 0, 48), ("dve", 48, 64)]
        DW2TV = [("dve", 16, 24)]
        DW2B = [("dve", 40, 64)]

        # ---------------- dyn blocks: dsc1 ----------------
        mid = [[None, None], [None, None]]
        y = [[None, None], [None, None]]
        g2t = [[None, None], [None, None]]
        nparts = [[None, None], [None, None]]
        cond1 = [None, None]

        deferred_relu = []

        def dw1_emit(i, c, bands, diag=None):
            m = new_img("big")
            ga = tiny.tile([128, 8], F32, tag="gacc")
            if diag is None:
                diag = make_diag(cond1[i][c])
            # x1's vec-band relus have ~25us of slack before cond2(x1);
            # defer them so they don't sit ahead of x2's diag builds on Act
            nparts[i][c] = dw_rows(xin[i][c], m, cond1[i][c], diag,
                                   bands, relu=True, gacc=ga,
                                   defer=deferred_relu if i == 0 else None)
            mid[i][c] = m
            g2t[i][c] = ga

        def flush_relu():
            for dst, lo, hi, ga, gi in deferred_relu:
                dv = pad2(dst[:])[:, 1 + lo: 1 + hi, 1: 65]
                nc.scalar.activation(dv, dv, AF.Relu, accum_out=ga[:, gi: gi + 1])
            deferred_relu.clear()

        gm_of(0)
        cond1[0] = conditioning(0, gms1[0])
        gm_of(1)  # x2 gm early: DVE/Act slots before x1's dw bands
        dw1_emit(0, 0, DW1_X1)
        cond1[1] = conditioning(0, gms1[1])  # pg/pw slot between x1 strip groups
        dw1_emit(0, 1, DW1_X1)
        load_w(["cw", "re1w", "re2w"])
        dw1_emit(1, 0, DW1_X2)
        dw1_emit(1, 1, DW1_X2)
        flush_relu()

        # ---------------- dsc2: top rows first, then bottom under the stages ----------------
        cond2 = [None, None]
        diag2 = [[None, None], [None, None]]

        def cond2_section(i):
            gms2 = []
            for c in range(2):
                g = tiny.tile([128, 1], F32, tag="gms")
                nc.vector.tensor_reduce(g[:], g2t[i][c][:, :nparts[i][c]],
                                        axis=mybir.AxisListType.X, op=AL.add)
                gms2.append(g)
            cond2[i] = conditioning(1, gms2)
            for c in range(2):
                yt = new_img("big")
                y[i][c] = yt
                diag2[i][c] = make_diag(cond2[i][c])
                dw_rows(mid[i][c], yt, cond2[i][c], diag2[i][c], DW2TV)

        def strip(s, i, c):
            dw_rows(mid[i][c], y[i][c], cond2[i][c], diag2[i][c],
                    [("pe", 8 * s, 8 * s + 8)])

        # x1's strips run on PE while gm2(x2) completes; cond2(x2)'s matmuls
        # would otherwise block the in-order PE queue
        cond2_section(0)
        strip(0, 0, 0)
        strip(0, 0, 1)
        cond2_section(1)

        def dsc2_bottom():
            for i in range(2):
                for c in range(2):
                    dw_rows(mid[i][c], y[i][c], cond2[i][c], diag2[i][c], DW2B)

        # ---------------- align 1x1 (2C -> C) -> fus (padded bf16) ----------------
        fus = [new_img("big") for _ in range(2)]

        def align_chunks(ns):
            for n in ns:
                for mc in range(2):
                    p = ps.tile([128, 512], F32, tag="ps")
                    for kc in range(4):
                        rhs = pad2(y[kc // 2][kc % 2][:])[:, 1 + n * 8: 9 + n * 8, 1: 65]
                        nc.tensor.matmul(
                            p[:], wt["alignw"][:, (kc * 2 + mc) * 128: (kc * 2 + mc + 1) * 128],
                            rhs, start=(kc == 0), stop=(kc == 3))
                    nc.scalar.activation(pad2(fus[mc][:])[:, 1 + n * 8: 9 + n * 8, 1: 65],
                                         p[:].rearrange("p (y x) -> p y x", y=8),
                                         AF.Identity, bias=wt["alignb"][:, mc: mc + 1])

        # ---------------- composed up conv (3x3, 256 -> 128 per phase) ----------------
        up2p = [new_img("big") for _ in range(4)]

        def up_chunks(ns):
            for n in ns:
                for p4 in range(4):
                    p = ps.tile([128, 512], F32, tag="ps")
                    first = True
                    for t9 in range(9):
                        dy, dx = t9 // 3, t9 % 3
                        for kc in range(2):
                            rhs = pad2(fus[kc][:])[:, n * 8 + dy: n * 8 + dy + 8,
                                                   dx: dx + 64]
                            nc.tensor.matmul(
                                p[:], wt["cw"][:, ((p4 * 9 + t9) * 2 + kc) * 128:
                                               ((p4 * 9 + t9) * 2 + kc + 1) * 128],
                                rhs, start=first, stop=(t9 == 8 and kc == 1))
                            first = False
                    nc.scalar.activation(pad2(up2p[p4][:])[:, 1 + n * 8: 9 + n * 8, 1: 65],
                                         p[:].rearrange("p (y x) -> p y x", y=8),
                                         AF.Identity, bias=wt["cb"][:, p4: p4 + 1])

        # pipeline: dw2 PE strips feed align/up chunk-by-chunk; bottom rows
        # run on DVE/Pool underneath the PE stage work
        strip(0, 1, 0)
        strip(0, 1, 1)
        align_chunks([0])
        for i in range(2):
            for c in range(2):
                strip(1, i, c)
        align_chunks([1])
        up_chunks([0])
        align_chunks([2])
        for i in range(2):
            for c in range(2):
                dw_rows(mid[i][c], y[i][c], cond2[i][c], diag2[i][c],
                        [("dve", 24, 40)])
        up_chunks([1])
        align_chunks([3, 4])
        up_chunks([2, 3])
        dsc2_bottom()
        align_chunks(range(5, 8))
        up_chunks(range(4, 8))

        # ---------------- re1 (polyphase 3x3, M-packed) ----------------
        re1t = new_img("big")
        for n in range(8):
            p = ps.tile([128, 512], F32, tag="ps")
            for ki, (pin, qy, qx) in enumerate(re1_keys):
                rhs = pad2(up2p[pin][:])[:, 1 + n * 8 + qy: 9 + n * 8 + qy,
                                         1 + qx: 65 + qx]
                nc.tensor.matmul(p[:], wt["re1w"][:, ki * 128: (ki + 1) * 128], rhs,
                                 start=(ki == 0), stop=(ki == len(re1_keys) - 1))
            nc.scalar.activation(pad2(re1t[:])[:, 1 + n * 8: 9 + n * 8, 1: 65],
                                 p[:].rearrange("p (y x) -> p y x", y=8),
                                 AF.Relu, bias=wt["re1b"][:])

        # ---------------- re2 (polyphase 3x3) + residual + interleave + out ----------------
        for n in range(8):
            for hb in range(2):  # half-bands of 8 output rows (4 phase rows)
                y0 = n * 8 + hb * 4
                sts = []
                for p4 in range(4):
                    p = ps.tile([128, 256], F32, tag="ps")
                    for qi, (qy, qx) in enumerate(re2_q[p4]):
                        rhs = pad2(re1t[:])[:, 1 + y0 + qy: 5 + y0 + qy,
                                            1 + qx: 65 + qx]
                        nc.tensor.matmul(p[:], wt["re2w"][:, (p4 * 4 + qi) * 128:
                                                          (p4 * 4 + qi + 1) * 128], rhs,
                                         start=(qi == 0), stop=(qi == len(re2_q[p4]) - 1))
                    st = stg.tile([128, 256], BF16, tag="stg")
                    nc.scalar.activation(st[:], p[:], AF.Identity, bias=wt["re2b"][:])
                    sts.append(st)
                band = bnd.tile([128, 1024], F32, tag="bnd")
                bv = band[:].rearrange("p (y r x s) -> p y r x s", y=4, r=2, s=2)
                for p4 in range(4):
                    eng = nc.vector if (p4 < 2 or (n == 7 and hb == 1)) else nc.gpsimd
                    r, s = p4 // 2, p4 % 2
                    up_v = pad2(up2p[p4][:])[:, 1 + y0: 5 + y0, 1: 65]
                    eng.tensor_tensor(
                        bv[:, :, r, :, s],
                        sts[p4][:].rearrange("p (y x) -> p y x", y=4),
                        up_v, AL.add)
                nc.sync.dma_start(
                    out_d.ap()[:, (2 * n + hb) * 1024: (2 * n + hb + 1) * 1024],
                    band[:])

    nc.compile()
    return nc


_NC = None


def _get_nc():
    global _NC
    if _NC is None:
        _NC = build()
    return _NC


def make_in_maps(inputs):
    w, _, _ = host_prep(inputs)
    xs = {}
    for nm in ("x1", "x2"):
        x = np.asarray(inputs[nm], np.float32).astype(BF_NP).reshape(NC, 256, 64, 64)
        xp = np.zeros((NC, 256, 64, 66), BF_NP)
        xp[:, :, :, 1:65] = x
        xs[nm] = xp.reshape(NC, 256, 4224)
    in_maps = []
    for i in range(NC):
        m = {"x1": np.ascontiguousarray(xs["x1"][i]),
             "x2": np.ascontiguousarray(xs["x2"][i])}
        m.update(w)
        in_maps.append(m)
    return in_maps


def kernel(**inputs):
    nc = _get_nc()
    in_maps = make_in_maps(inputs)
    res = run_bass_kernel_spmd(nc, in_maps, core_ids=list(range(NC)))
    out = np.stack([res.results[i]["out"].reshape(128, 128, 128) for i in range(NC)])
    return out.astype(np.float32)
